# revision 1
# baseline (speedup 1.0000x reference)
"""Trainium2 Bass kernel for nn_ContinuousEmbedding (masked matmul + bias).

Computes out = x @ (weights * mask) + bias, reshaped to [B, in_size, out_size],
where mask zeroes each input feature's own [out_size]-wide diagonal block.

Strategy: tensor-parallel across the 8 NeuronCores by splitting the
in_size*out_size (=16384) output columns into 8 shards of 2048 columns.
Each core gets the full x (as x^T for the matmul's stationary operand),
its weight-column shard (mask is constant — folded into the weights on the
host), and its bias shard. Per core: out_shard = x @ W_shard + bias_shard
via 128x128 fp32 matmuls accumulating K=256 as 2 PSUM passes; bias-add is
fused into the PSUM->SBUF eviction on the vector engine.
"""

import numpy as np

B = 4096
IN_SIZE = 256
OUT_SIZE = 64
IO = IN_SIZE * OUT_SIZE          # 16384
N_CORES = 8
N_SHARD = IO // N_CORES          # 2048 output columns per core
P = 128                          # SBUF partitions
KO = IN_SIZE // P                # 2 contraction sub-tiles
N_TILE = 512                     # matmul moving free dim (fp32 max)
M_TILES = B // P                 # 32 output row tiles
N_TILES = N_SHARD // N_TILE      # 4 column tiles per core

MATMUL_MODE = "fp32r"            # "fp32" | "fp32r" | "fp32x3"

_CACHE: dict = {}


def _build_program(mode=None):
    mode = mode or MATMUL_MODE
    import concourse.mybir as mybir
    import concourse.tile as tile
    import concourse.bass as bass
    from concourse import bacc

    nsplit = 2 if mode == "fp32x3" else 1
    nc = bacc.Bacc(
        "TRN2", target_bir_lowering=False, debug=False, num_devices=N_CORES
    )
    # fp32r modes: operands are pre-rounded on the host to fp32r's 12
    # significand bits (round-to-nearest-even), so the DRAM tensors are
    # declared float32r and plain (non-casting) HWDGE DMAs load them.
    in_dt = mybir.dt.float32 if mode == "fp32" else mybir.dt.float32r
    xt = nc.dram_tensor(
        "xt", [nsplit, IN_SIZE, B], in_dt, kind="ExternalInput"
    ).ap()
    w = nc.dram_tensor(
        "w", [nsplit, IN_SIZE, N_SHARD], in_dt, kind="ExternalInput"
    ).ap()
    bias = nc.dram_tensor(
        "bias", [N_SHARD], mybir.dt.float32, kind="ExternalInput"
    ).ap()
    out = nc.dram_tensor(
        "out", [B, N_SHARD], mybir.dt.float32, kind="ExternalOutput"
    ).ap()

    with tile.TileContext(nc) as tc:
        with tc.tile_pool(name="const", bufs=1) as const, \
             tc.tile_pool(name="psum", bufs=2, space="PSUM") as psum_pool, \
             tc.tile_pool(name="outp", bufs=4) as outp:
            mm_dt = (mybir.dt.float32 if mode == "fp32"
                     else mybir.dt.float32r)
            w_sb = const.tile([P, nsplit, KO, N_SHARD], mm_dt)
            xt_sb = const.tile([P, nsplit, KO, B], mm_dt)
            bias_sb = const.tile([P, N_SHARD], mybir.dt.float32)

            # Whole-tensor DMAs keep the per-partition packets large
            # (8-16KB) — fragmented loads pay per-packet overhead.
            ld_eng = nc.sync
            w_src = w.rearrange("s (ko p) n -> p s ko n", p=P)
            ld_eng.dma_start(out=w_sb[:], in_=w_src[:])
            # x^T load: first chunk smaller so m-tile 0 starts sooner.
            xt_src = xt.rearrange("s (ko p) m -> p s ko m", p=P)
            for lo, hi in [(0, 1024), (1024, B)]:
                sl = slice(lo, hi)
                ld_eng.dma_start(out=xt_sb[:, :, :, sl], in_=xt_src[:, :, :, sl])
            # bias [N_SHARD] broadcast across all 128 partitions (stride-0
            # DRAM read).
            bias_bcast = bass.AP(
                tensor=bias.tensor,
                offset=bias.offset,
                ap=[[0, P]] + list(bias.ap),
            )
            ld_eng.dma_start(out=bias_sb[:], in_=bias_bcast)

            # (x_split, w_split) matmul terms: plain modes use (0,0);
            # fp32x3 adds the hi/lo cross terms (lo@hi, hi@lo).
            terms = [(0, 0)] if nsplit == 1 else [(0, 0), (1, 0), (0, 1)]
            for m in range(M_TILES):
                out_sb = outp.tile([P, N_SHARD], mybir.dt.float32)
                for n in range(N_TILES):
                    ns = slice(n * N_TILE, (n + 1) * N_TILE)
                    ps = psum_pool.tile([P, N_TILE], mybir.dt.float32,
                                        name=f"ps{n}", tag=f"ps{n}")
                    nmm = KO * len(terms)
                    for i, (k, (xi, wi)) in enumerate(
                        (k, t) for k in range(KO) for t in terms
                    ):
                        nc.tensor.matmul(
                            ps[:],
                            lhsT=xt_sb[:, xi, k, m * P:(m + 1) * P],
                            rhs=w_sb[:, wi, k, ns],
                            start=(i == 0),
                            stop=(i == nmm - 1),
                        )
                    nc.vector.tensor_add(out_sb[:, ns], ps[:], bias_sb[:, ns])
                nc.sync.dma_start(out=out[m * P:(m + 1) * P, :], in_=out_sb[:])

    nc.compile()
    return nc


def _get_program(mode=None):
    mode = mode or MATMUL_MODE
    if mode not in _CACHE:
        _CACHE[mode] = _build_program(mode)
    return _CACHE[mode]


def _round12(a):
    """Round fp32 to fp32r's 12 significand bits (round-to-nearest-even)."""
    u = a.view(np.uint32)
    r = (u + np.uint32(0x7FF) + ((u >> np.uint32(12)) & np.uint32(1)))
    return (r & np.uint32(0xFFFFF000)).view(np.float32)


def _hi_lo(a):
    hi = _round12(a)
    return np.stack([hi, _round12(a - hi)], axis=0)


def _shard_inputs(x, weights, bias, mode=None):
    mode = mode or MATMUL_MODE
    # Fold the constant block-diagonal mask into the weights on the host.
    col_block = np.arange(IO, dtype=np.int64) // OUT_SIZE
    mask = (col_block[None, :] != np.arange(IN_SIZE)[:, None])
    wm = weights * mask.astype(weights.dtype)
    xt = np.ascontiguousarray(x.T)
    if mode == "fp32x3":
        xt_in = _hi_lo(xt)
    elif mode == "fp32r":
        xt_in = _round12(xt)[None]
    else:
        xt_in = xt[None]
    in_maps = []
    for c in range(N_CORES):
        sl = slice(c * N_SHARD, (c + 1) * N_SHARD)
        w_shard = np.ascontiguousarray(wm[:, sl])
        if mode == "fp32x3":
            w_in = _hi_lo(w_shard)
        elif mode == "fp32r":
            w_in = _round12(w_shard)[None]
        else:
            w_in = w_shard[None]
        in_maps.append({
            "xt": xt_in,
            "w": np.ascontiguousarray(w_in),
            "bias": np.ascontiguousarray(bias[sl]),
        })
    return in_maps


def run_sharded(in_maps, mode=None, **kwargs):
    """Run the SPMD program on cores 0-7. kwargs forwarded (e.g. trace)."""
    from concourse.bass_utils import run_bass_kernel_spmd

    nc = _get_program(mode)
    return run_bass_kernel_spmd(
        nc, in_maps, core_ids=list(range(N_CORES)), **kwargs
    )


def kernel(x: np.ndarray, weights: np.ndarray, bias: np.ndarray) -> np.ndarray:
    x = np.asarray(x, dtype=np.float32)
    weights = np.asarray(weights, dtype=np.float32)
    bias = np.asarray(bias, dtype=np.float32)
    in_maps = _shard_inputs(x, weights, bias)
    res = run_sharded(in_maps)
    full = np.concatenate([res.results[c]["out"] for c in range(N_CORES)], axis=1)
    return full.reshape(B, IN_SIZE, OUT_SIZE)



# revision 5
# speedup vs baseline: 1.5992x; 1.5992x over previous
"""Trainium2 Bass kernel for nn_ContinuousEmbedding (masked matmul + bias).

Computes out = x @ (weights * mask) + bias, reshaped to [B, in_size, out_size],
where mask zeroes each input feature's own [out_size]-wide diagonal block.

Strategy: tensor-parallel across the 8 NeuronCores by splitting the
in_size*out_size (=16384) output columns into 8 shards of 2048 columns.
The mask is constant and folded into the weights on the host.

v2 (all-bf16, transposed output): everything is cast to bf16 on the host
(tolerance is 2e-2 rel-l2; bf16 end-to-end costs ~2e-3), halving both the
input loads (3 MB/core) and the output stores (16 MB/core) so the DMA time
(~19 MB / ~360 GB/s ~= 53 us) sits right at the PE floor (256 matmuls x 512
rows @ 2.4 GHz = 54.6 us).  The matmul is transposed vs v1: the stationary
operand is a W column-tile, the moving operand is x^T, so PSUM tiles are
[128 out-cols, 512 batch].  With output COLUMNS on the partition axis the
bias becomes a per-partition scalar, which lets both element-wise engines
that can read PSUM (DVE tensor_scalar_add, Act activation-Identity) share
the PSUM->SBUF eviction with the bias add and the fp32->bf16 cast fused
(~0.6 us per [128,512] tile, alternating; GPSIMD/Pool cannot access PSUM
on TRN2).  Stores go through three DMA rings -- SP and Act HWDGE plus
Pool SWDGE -- ~6-7 MB each.  The host transposes the gathered
[16384, 4096] bf16 result back and upcasts to fp32.
"""

import numpy as np
import ml_dtypes

B = 4096
IN_SIZE = 256
OUT_SIZE = 64
IO = IN_SIZE * OUT_SIZE          # 16384
N_CORES = 8
N_SHARD = IO // N_CORES          # 2048 output columns per core
P = 128                          # SBUF partitions
KO = IN_SIZE // P                # 2 contraction sub-tiles
M_TILE = 512                     # matmul moving free dim (= PSUM bank, fp32)
M_CHUNKS = B // M_TILE           # 8 batch chunks
NT = N_SHARD // P                # 16 column tiles (out partitions) per core

BF16 = np.dtype(ml_dtypes.bfloat16)

_CACHE: dict = {}


def _build_program():
    import concourse.mybir as mybir
    import concourse.tile as tile
    from concourse import bacc

    nc = bacc.Bacc(
        "TRN2", target_bir_lowering=False, debug=False, num_devices=N_CORES
    )
    bf = mybir.dt.bfloat16
    f32 = mybir.dt.float32
    xt = nc.dram_tensor("xt", [KO, P, B], bf, kind="ExternalInput").ap()
    w = nc.dram_tensor("w", [KO, P, N_SHARD], bf, kind="ExternalInput").ap()
    # bias pre-transposed on host to [P, NT] (partition-major).
    bias = nc.dram_tensor("bias", [P, NT], f32, kind="ExternalInput").ap()
    # out^T: [n_cols, batch]; host transposes back.
    out = nc.dram_tensor("out", [N_SHARD, B], bf, kind="ExternalOutput").ap()

    with tile.TileContext(nc) as tc:
        with tc.tile_pool(name="const", bufs=1) as const, \
             tc.tile_pool(name="psum", bufs=8, space="PSUM") as psum_pool, \
             tc.tile_pool(name="stage", bufs=2) as stage_pool:
            xt_sb = const.tile([P, KO, B], bf)
            w_sb = const.tile([P, KO, N_SHARD], bf)
            bias_sb = const.tile([P, NT], f32)

            # Loads. SP ring carries W (in 4 column chunks so the first
            # matmuls can start after ~256KB); Act ring carries bias + x^T
            # (first 512-batch chunk alone for an early PE start, then two
            # coarser pieces).  Per-partition runs are 1-4KB so HWDGE
            # descriptors stay efficient.
            w_src = w.rearrange("k p n -> p k n")
            for i in range(4):
                cs = slice(i * (N_SHARD // 4), (i + 1) * (N_SHARD // 4))
                nc.sync.dma_start(out=w_sb[:, :, cs], in_=w_src[:, :, cs])
            xt_src = xt.rearrange("k p m -> p k m")
            for lo, hi in [(0, 512), (512, 2048), (2048, B)]:
                ms = slice(lo, hi)
                nc.scalar.dma_start(out=xt_sb[:, :, ms], in_=xt_src[:, :, ms])
            nc.scalar.dma_start(out=bias_sb[:], in_=bias[:])

            out_r = out.rearrange("(t p) m -> p t m", p=P)
            for m in range(M_CHUNKS):
                ms = slice(m * M_TILE, (m + 1) * M_TILE)
                stage = stage_pool.tile([P, NT, M_TILE], bf)
                for t in range(NT):
                    ns = slice(t * P, (t + 1) * P)
                    ps = psum_pool.tile([P, M_TILE], f32)
                    nc.tensor.matmul(
                        ps[:], lhsT=w_sb[:, 0, ns], rhs=xt_sb[:, 0, ms],
                        start=True, stop=False,
                    )
                    nc.tensor.matmul(
                        ps[:], lhsT=w_sb[:, 1, ns], rhs=xt_sb[:, 1, ms],
                        start=False, stop=True,
                    )
                    # PSUM->SBUF eviction with fused bias add + bf16 cast,
                    # alternating DVE / Act (Pool cannot read PSUM on TRN2).
                    dst = stage[:, t, :]
                    bs = bias_sb[:, t:t + 1]
                    if t % 2 == 0:
                        nc.vector.tensor_scalar_add(dst, ps[:], bs)
                    else:
                        nc.scalar.activation(
                            dst, ps[:],
                            mybir.ActivationFunctionType.Identity,
                            bias=bs, scale=1.0,
                        )
                # Store the chunk through three DMA rings: SP + Act HWDGE
                # and Pool SWDGE (stores read SBUF, which Pool may access).
                nc.sync.dma_start(out=out_r[:, 0:6, ms], in_=stage[:, 0:6, :])
                nc.scalar.dma_start(
                    out=out_r[:, 6:10, ms], in_=stage[:, 6:10, :]
                )
                nc.gpsimd.dma_start(
                    out=out_r[:, 10:NT, ms], in_=stage[:, 10:NT, :]
                )

    nc.compile()
    return nc


def _get_program(mode=None):
    if "prog" not in _CACHE:
        _CACHE["prog"] = _build_program()
    return _CACHE["prog"]


def _shard_inputs(x, weights, bias, mode=None):
    # Fold the constant block-diagonal mask into the weights on the host.
    col_block = np.arange(IO, dtype=np.int64) // OUT_SIZE
    mask = (col_block[None, :] != np.arange(IN_SIZE)[:, None])
    wm = weights * mask.astype(weights.dtype)
    xt16 = x.T.astype(BF16).reshape(KO, P, B)
    in_maps = []
    for c in range(N_CORES):
        sl = slice(c * N_SHARD, (c + 1) * N_SHARD)
        w16 = wm[:, sl].astype(BF16).reshape(KO, P, N_SHARD)
        bias_t = np.ascontiguousarray(
            bias[sl].astype(np.float32).reshape(NT, P).T
        )
        in_maps.append({
            "xt": xt16,
            "w": np.ascontiguousarray(w16),
            "bias": bias_t,
        })
    return in_maps


def run_sharded(in_maps, mode=None, **kwargs):
    """Run the SPMD program on cores 0-7. kwargs forwarded (e.g. trace)."""
    from concourse.bass_utils import run_bass_kernel_spmd

    nc = _get_program()
    return run_bass_kernel_spmd(
        nc, in_maps, core_ids=list(range(N_CORES)), **kwargs
    )


def kernel(x: np.ndarray, weights: np.ndarray, bias: np.ndarray) -> np.ndarray:
    x = np.asarray(x, dtype=np.float32)
    weights = np.asarray(weights, dtype=np.float32)
    bias = np.asarray(bias, dtype=np.float32)
    in_maps = _shard_inputs(x, weights, bias)
    res = run_sharded(in_maps)
    # Each core returns out^T [N_SHARD, B] bf16; transpose back and upcast.
    full = np.concatenate(
        [np.asarray(res.results[c]["out"]).T for c in range(N_CORES)], axis=1
    ).astype(np.float32)
    return full.reshape(B, IN_SIZE, OUT_SIZE)


# revision 6
# speedup vs baseline: 1.6221x; 1.0143x over previous
"""Trainium2 Bass kernel for nn_ContinuousEmbedding (masked matmul + bias).

Computes out = x @ (weights * mask) + bias, reshaped to [B, in_size, out_size],
where mask zeroes each input feature's own [out_size]-wide diagonal block.

Strategy: tensor-parallel across the 8 NeuronCores by splitting the
in_size*out_size (=16384) output columns into 8 shards of 2048 columns.
The mask is constant and folded into the weights on the host.

All-bf16, transposed output: everything is cast to bf16 on the host
(tolerance is 2e-2 rel-l2; bf16 end-to-end costs ~3e-3), halving both the
input loads (3 MB/core) and the output stores (16 MB/core) so the DMA time
(~19 MB / ~330 GB/s) sits right at the PE floor (256 matmuls x 512 rows
@ 2.4 GHz = 54.6 us).  The matmul is transposed: the stationary operand is
a W column-tile, the moving operand is x^T, so PSUM tiles hold
[128 out-cols, batch].  With output COLUMNS on the partition axis the bias
is a per-partition scalar, so both PSUM-capable element-wise engines (DVE
tensor_scalar_add, Act activation-Identity) evict PSUM->SBUF with the bias
add and fp32->bf16 cast fused.

v4 refinements (from NTFF traces):
 - Batch chunks are processed in PAIRS (two 512-col matmuls per stationary
   LDWEIGHTS), halving weight-register reloads (~3 us of PE time).
 - PSUM tiles are [128, 1024] (2 banks, 4 bufs): halves the eviction
   instruction count (64 x ~1.2 us, alternating DVE/Act) and the semaphore
   count -- the TileContext epilogue resets every semaphore serially at
   ~115 ns each, so fewer sems directly shrinks the fixed tail.
 - Stores ride three DMA rings (SP + Act HWDGE, Pool SWDGE); the final
   m-pair's stores are split into 8 small DMAs so the last transfer does
   not leave a multi-us tail.
 - First x^T piece is small and the second rides the SP ring so the first
   matmuls start as early as the prologue allows.
"""

import numpy as np
import ml_dtypes

B = 4096
IN_SIZE = 256
OUT_SIZE = 64
IO = IN_SIZE * OUT_SIZE          # 16384
N_CORES = 8
N_SHARD = IO // N_CORES          # 2048 output columns per core
P = 128                          # SBUF partitions
KO = IN_SIZE // P                # 2 contraction sub-tiles
M_TILE = 512                     # matmul moving free dim (= PSUM bank, fp32)
M_PAIR = 2 * M_TILE              # 1024: two matmuls per stationary load
M_PAIRS = B // M_PAIR            # 4 batch pair-chunks
NT = N_SHARD // P                # 16 column tiles (out partitions) per core

BF16 = np.dtype(ml_dtypes.bfloat16)

_CACHE: dict = {}


def _build_program():
    import concourse.mybir as mybir
    import concourse.tile as tile
    from concourse import bacc

    nc = bacc.Bacc(
        "TRN2", target_bir_lowering=False, debug=False, num_devices=N_CORES
    )
    bf = mybir.dt.bfloat16
    f32 = mybir.dt.float32
    xt = nc.dram_tensor("xt", [KO, P, B], bf, kind="ExternalInput").ap()
    w = nc.dram_tensor("w", [KO, P, N_SHARD], bf, kind="ExternalInput").ap()
    # bias pre-transposed on host to [P, NT] (partition-major).
    bias = nc.dram_tensor("bias", [P, NT], f32, kind="ExternalInput").ap()
    # out^T: [n_cols, batch]; host transposes back.
    out = nc.dram_tensor("out", [N_SHARD, B], bf, kind="ExternalOutput").ap()

    with tile.TileContext(nc) as tc:
        with tc.tile_pool(name="const", bufs=1) as const, \
             tc.tile_pool(name="psum", bufs=4, space="PSUM") as psum_pool, \
             tc.tile_pool(name="stage", bufs=2) as stage_pool:
            xt_sb = const.tile([P, KO, B], bf)
            w_sb = const.tile([P, KO, N_SHARD], bf)
            bias_sb = const.tile([P, NT], f32)

            # Loads.  SP ring: W column-chunks (first chunk first so the
            # first LDWEIGHTS fires ASAP) with the second x^T piece
            # interleaved; Act ring: first x^T piece, bias, rest of x^T.
            # Per-partition runs are 1-4KB so HWDGE descriptors stay fat.
            w_src = w.rearrange("k p n -> p k n")
            xt_src = xt.rearrange("k p m -> p k m")
            wc = N_SHARD // 4
            nc.sync.dma_start(out=w_sb[:, :, 0:wc], in_=w_src[:, :, 0:wc])
            nc.scalar.dma_start(
                out=xt_sb[:, :, 0:512], in_=xt_src[:, :, 0:512]
            )
            nc.sync.dma_start(
                out=xt_sb[:, :, 512:1024], in_=xt_src[:, :, 512:1024]
            )
            nc.scalar.dma_start(out=bias_sb[:], in_=bias[:])
            for i in range(1, 4):
                cs = slice(i * wc, (i + 1) * wc)
                nc.sync.dma_start(out=w_sb[:, :, cs], in_=w_src[:, :, cs])
            for lo, hi in [(1024, 2560), (2560, B)]:
                ms = slice(lo, hi)
                nc.scalar.dma_start(out=xt_sb[:, :, ms], in_=xt_src[:, :, ms])

            out_r = out.rearrange("(t p) m -> p t m", p=P)
            for m in range(M_PAIRS):
                ms0 = slice(m * M_PAIR, m * M_PAIR + M_TILE)
                ms1 = slice(m * M_PAIR + M_TILE, (m + 1) * M_PAIR)
                ms = slice(m * M_PAIR, (m + 1) * M_PAIR)
                stage = stage_pool.tile([P, NT, M_PAIR], bf)
                for t in range(NT):
                    ns = slice(t * P, (t + 1) * P)
                    ps = psum_pool.tile([P, M_PAIR], f32)
                    # One LDWEIGHTS per (k, t); two 512-row matmuls reuse it.
                    for k in range(KO):
                        st, sp = (k == 0), (k == KO - 1)
                        nc.tensor.matmul(
                            ps[:, 0:M_TILE], lhsT=w_sb[:, k, ns],
                            rhs=xt_sb[:, k, ms0], start=st, stop=sp,
                        )
                        nc.tensor.matmul(
                            ps[:, M_TILE:M_PAIR], lhsT=w_sb[:, k, ns],
                            rhs=xt_sb[:, k, ms1], start=st, stop=sp,
                        )
                    # PSUM->SBUF eviction with fused bias add + bf16 cast,
                    # alternating DVE / Act (Pool cannot read PSUM on TRN2).
                    dst = stage[:, t, :]
                    bs = bias_sb[:, t:t + 1]
                    if t % 2 == 0:
                        nc.vector.tensor_scalar_add(dst, ps[:], bs)
                    else:
                        nc.scalar.activation(
                            dst, ps[:],
                            mybir.ActivationFunctionType.Identity,
                            bias=bs, scale=1.0,
                        )
                # Stores ride three rings: SP + Act HWDGE, Pool SWDGE.
                # The final pair-chunk is split fine so the very last DMA
                # is small (short tail after the last eviction).
                if m < M_PAIRS - 1:
                    splits = [(0, 5, nc.sync), (5, 9, nc.scalar),
                              (9, NT, nc.gpsimd)]
                else:
                    splits = [(0, 2, nc.sync), (2, 4, nc.scalar),
                              (4, 6, nc.gpsimd), (6, 8, nc.sync),
                              (8, 10, nc.scalar), (10, 12, nc.gpsimd),
                              (12, 14, nc.sync), (14, NT, nc.scalar)]
                for lo, hi, eng in splits:
                    eng.dma_start(
                        out=out_r[:, lo:hi, ms], in_=stage[:, lo:hi, :]
                    )

    nc.compile()
    return nc


def _get_program(mode=None):
    if "prog" not in _CACHE:
        _CACHE["prog"] = _build_program()
    return _CACHE["prog"]


def _shard_inputs(x, weights, bias, mode=None):
    # Fold the constant block-diagonal mask into the weights on the host.
    col_block = np.arange(IO, dtype=np.int64) // OUT_SIZE
    mask = (col_block[None, :] != np.arange(IN_SIZE)[:, None])
    wm = weights * mask.astype(weights.dtype)
    xt16 = x.T.astype(BF16).reshape(KO, P, B)
    in_maps = []
    for c in range(N_CORES):
        sl = slice(c * N_SHARD, (c + 1) * N_SHARD)
        w16 = wm[:, sl].astype(BF16).reshape(KO, P, N_SHARD)
        bias_t = np.ascontiguousarray(
            bias[sl].astype(np.float32).reshape(NT, P).T
        )
        in_maps.append({
            "xt": xt16,
            "w": np.ascontiguousarray(w16),
            "bias": bias_t,
        })
    return in_maps


def run_sharded(in_maps, mode=None, **kwargs):
    """Run the SPMD program on cores 0-7. kwargs forwarded (e.g. trace)."""
    from concourse.bass_utils import run_bass_kernel_spmd

    nc = _get_program()
    return run_bass_kernel_spmd(
        nc, in_maps, core_ids=list(range(N_CORES)), **kwargs
    )


def kernel(x: np.ndarray, weights: np.ndarray, bias: np.ndarray) -> np.ndarray:
    x = np.asarray(x, dtype=np.float32)
    weights = np.asarray(weights, dtype=np.float32)
    bias = np.asarray(bias, dtype=np.float32)
    in_maps = _shard_inputs(x, weights, bias)
    res = run_sharded(in_maps)
    # Each core returns out^T [N_SHARD, B] bf16; transpose back and upcast.
    full = np.concatenate(
        [np.asarray(res.results[c]["out"]).T for c in range(N_CORES)], axis=1
    ).astype(np.float32)
    return full.reshape(B, IN_SIZE, OUT_SIZE)
